# revision 1
# baseline (speedup 1.0000x reference)
"""Distance-aware transformer encoder layer on 8 Trainium2 NeuronCores.

Sharding: core c handles batch b = c//2 and query-half qh = c%2 (512 query
rows). K/V are computed per-core for the full 1024-key sequence of its batch
(duplicated across the core pair — cheaper than a collective). Everything
else (scores, softmax, out-proj, LayerNorms, FFN) is perfectly sharded by
query rows. No collectives.

Precision: fp32r (full-rate TF32-like PE mode) for the score path
(Q/K projections, distance bias, FFN mm1); bf16 for E=exp(scores), V,
attn-out projection, h=relu(...), and FFN mm2. LayerNorm statistics and
residuals in fp32. Measured end-to-end ~1.3e-3 scale-relative absmax vs
the fp32 reference.
"""

import numpy as np
import ml_dtypes

import bass_rust
import concourse.bass as bass
import concourse.tile as tile
import concourse.mybir as mybir
from concourse.bass import AP
from concourse.bass_utils import run_bass_kernel_spmd

B, S, D, H, DFF, HD = 4, 1024, 1024, 16, 4096, 64
SQ = 512          # query rows per core
NCORES = 8
EPS = 1e-5
F32 = mybir.dt.float32
F32R = mybir.dt.float32r
BF16 = mybir.dt.bfloat16
FT = mybir.ActivationFunctionType
ALU = mybir.AluOpType

_nop_ctr = [0]


def _legalize_waits(nc):
    """walrus codegen in this toolchain accepts only one sync-wait per
    instruction; split extras onto same-engine NoOps inserted before."""
    n_fixed = 0
    for f in nc.m.functions:
        for bb in f.blocks:
            insts = bb.instructions
            i = 0
            while i < len(insts):
                inst = insts[i]
                si = inst.sync_info
                waits = list(si.on_wait) if si is not None and si.on_wait else []
                if len(waits) > 1:
                    keep = waits[-1]
                    for w in waits[:-1]:
                        n = bass_rust.InstNoOp(
                            name=f"waitsplit-nop-{_nop_ctr[0]}", ins=[], outs=[]
                        )
                        _nop_ctr[0] += 1
                        n.engine = inst.engine
                        n.sync_info = bass_rust.SyncInfo(on_update=[], on_wait=[w])
                        insts.insert(i, n)
                        i += 1
                    inst.sync_info = bass_rust.SyncInfo(
                        on_update=list(si.on_update or []), on_wait=[keep]
                    )
                    n_fixed += 1
                i += 1
    return n_fixed


def _bcast_ap(dram_ap, reps):
    """Broadcast a DRAM AP over leading zero-step dims of sizes `reps`."""
    return AP(
        tensor=dram_ap.tensor,
        offset=dram_ap.offset,
        ap=[[0, r] for r in reps] + list(dram_ap.ap),
    )


def _build():
    nc = bass.Bass()
    dp = nc.declare_dram_parameter

    SrcT = dp("srcT", [D, S], F32R, isOutput=False)        # src[b][perm].T
    SrcQ = dp("src_q", [SQ, D], F32, isOutput=False)       # natural query rows
    DistT = dp("distT", [S, SQ], F32, isOutput=False)      # distances[q,:].T
    Wq = dp("Wq", [D, D], F32R, isOutput=False)            # pre-scaled by HD^-0.5
    Wk = dp("Wk", [D, D], F32R, isOutput=False)
    Wv = dp("Wv", [D, D], F32R, isOutput=False)
    Wo = dp("Wo", [D, D], BF16, isOutput=False)
    W1 = dp("W1", [D, DFF], F32R, isOutput=False)
    W2 = dp("W2", [DFF, D], BF16, isOutput=False)
    Bq2 = dp("bq2d", [128, 8], F32, isOutput=False)        # (bq*scale) tiled
    Bk2 = dp("bk2d", [128, 8], F32, isOutput=False)
    B12 = dp("b1_2d", [128, 32], F32, isOutput=False)
    BvR = dp("bv_r", [1, D], F32R, isOutput=False)
    B2R = dp("b2_r", [1, D], F32R, isOutput=False)
    G1 = dp("g1v", [D], BF16, isOutput=False)
    Be1 = dp("beta1v", [D], BF16, isOutput=False)
    G2 = dp("g2v", [D], BF16, isOutput=False)
    Be2 = dp("beta2v", [D], BF16, isOutput=False)
    NegA = dp("negabs", [1, 1], F32, isOutput=False)       # -|dist_scale|
    Ident = dp("ident", [128, 128], F32R, isOutput=False)
    OnesR = dp("ones_row", [1, 512], F32R, isOutput=False)
    OnesB = dp("ones_bf", [1, 1], BF16, isOutput=False)
    Out = dp("out", [SQ, D], F32, isOutput=True)

    with tile.TileContext(nc) as tc:
        import contextlib

        ctx = contextlib.ExitStack()
        with ctx:
            consts = ctx.enter_context(tc.tile_pool(name="consts", bufs=1))
            arena4 = ctx.enter_context(tc.tile_pool(name="arena4", bufs=1))
            arena4b = ctx.enter_context(tc.tile_pool(name="arena4b", bufs=1))
            arena2 = ctx.enter_context(tc.tile_pool(name="arena2", bufs=1))
            arena2b = ctx.enter_context(tc.tile_pool(name="arena2b", bufs=1))
            persist = ctx.enter_context(tc.tile_pool(name="persist", bufs=1))
            wpool = ctx.enter_context(tc.tile_pool(name="wpool", bufs=2))
            epool = ctx.enter_context(tc.tile_pool(name="epool", bufs=3))
            lnpool = ctx.enter_context(tc.tile_pool(name="lnpool", bufs=1))
            small = ctx.enter_context(tc.tile_pool(name="small", bufs=2))
            ps_s = ctx.enter_context(tc.tile_pool(name="ps_s", bufs=2, space="PSUM"))
            dscratch = ctx.enter_context(tc.tile_pool(name="dscratch", bufs=4, space="DRAM"))
            ps_mm = ctx.enter_context(tc.tile_pool(name="ps_mm", bufs=3, space="PSUM"))

            # ---- constants ----
            ident = consts.tile([128, 128], F32R, tag="ident")
            nc.sync.dma_start(out=ident, in_=Ident[:, :])
            spin = ps_mm.tile([128, 512], F32, tag="psmm")
            for _ in range(160):
                nc.tensor.matmul(spin[:, 0:128], ident, ident, start=True, stop=True)
            ones_row = consts.tile([1, 512], F32R, tag="ones_row")
            nc.sync.dma_start(out=ones_row, in_=OnesR[:, :])
            bq2 = consts.tile([128, 8], F32, tag="bq2")
            nc.sync.dma_start(out=bq2, in_=Bq2[:, :])
            bk2 = consts.tile([128, 8], F32, tag="bk2")
            nc.sync.dma_start(out=bk2, in_=Bk2[:, :])
            b12 = consts.tile([128, 32], F32, tag="b12")
            nc.sync.dma_start(out=b12, in_=B12[:, :])
            bv_r = consts.tile([1, D], F32R, tag="bv_r")
            nc.sync.dma_start(out=bv_r, in_=BvR[:, :])
            b2_r = consts.tile([1, D], F32R, tag="b2_r")
            nc.sync.dma_start(out=b2_r, in_=B2R[:, :])
            g1b = consts.tile([128, D], BF16, tag="gb")
            nc.sync.dma_start(out=g1b, in_=_bcast_ap(G1[:], [128]))
            be1b = consts.tile([128, D], BF16, tag="beb")
            nc.sync.dma_start(out=be1b, in_=_bcast_ap(Be1[:], [128]))
            negabs = consts.tile([128, 1], F32, tag="negabs")
            nc.sync.dma_start(out=negabs, in_=_bcast_ap(NegA[0, 0:1], [128]))
            log_eps = consts.tile([128, 1], F32, tag="log_eps")
            nc.vector.memset(log_eps, 1e-9)
            ln_eps = consts.tile([128, 1], F32, tag="ln_eps")
            nc.vector.memset(ln_eps, EPS)

            # ---- distance bias: biasT[k, q] = -|s| * ln(dist^T + 1e-9) ----
            biasT = persist.tile([128, 8, 512], F32R, tag="biasT")
            for ko in range(8):
                dtile = lnpool.tile([128, 1024], F32, tag="lnbig")
                nc.sync.dma_start(
                    out=dtile[:, 0:512], in_=DistT[ko * 128 : ko * 128 + 128, :]
                )
                lnt = lnpool.tile([128, 1024], F32, tag="lnbig2")
                nc.scalar.activation(
                    out=lnt[:, 0:512], in_=dtile[:, 0:512], func=FT.Ln, bias=log_eps
                )
                nc.vector.tensor_scalar_mul(
                    out=biasT[:, ko, :], in0=lnt[:, 0:512], scalar1=negabs
                )

            # ---- srcT resident ----
            srcT = arena4.tile([128, 8, 1024], F32R, tag="a4")
            for co in range(8):
                nc.sync.dma_start(
                    out=srcT[:, co, :], in_=SrcT[co * 128 : co * 128 + 128, :]
                )

            # ---- K^T projection: kt[dout, keys], fp32r ----
            kt = arena4b.tile([128, 8, 1024], F32R, tag="a4b")
            for wc in range(4):
                wbuf = wpool.tile([128, 8, 256], F32R, tag="wbuf")
                nc.sync.dma_start(
                    out=wbuf,
                    in_=Wk[:, wc * 256 : wc * 256 + 256].rearrange(
                        "(ko ki) m -> ki ko m", ki=128
                    ),
                )
                for dl in range(2):
                    dt = wc * 2 + dl
                    for nt in range(2):
                        psum = ps_mm.tile([128, 512], F32, tag="psmm")
                        for ko in range(8):
                            nc.tensor.matmul(
                                psum,
                                wbuf[:, ko, dl * 128 : dl * 128 + 128],
                                srcT[:, ko, nt * 512 : nt * 512 + 512],
                                start=(ko == 0),
                                stop=(ko == 7),
                            )
                        nc.vector.tensor_scalar_add(
                            out=kt[:, dt, nt * 512 : nt * 512 + 512],
                            in0=psum,
                            scalar1=bk2[:, dt : dt + 1],
                        )

            # ---- Q^T projection: qt[dout, q], fp32r (Wq pre-scaled) ----
            qt = arena2.tile([128, 8, 512], F32R, tag="a2")
            for wc in range(4):
                wbuf = wpool.tile([128, 8, 256], F32R, tag="wbuf")
                nc.sync.dma_start(
                    out=wbuf,
                    in_=Wq[:, wc * 256 : wc * 256 + 256].rearrange(
                        "(ko ki) m -> ki ko m", ki=128
                    ),
                )
                for dl in range(2):
                    dt = wc * 2 + dl
                    psum = ps_mm.tile([128, 512], F32, tag="psmm")
                    for ko in range(8):
                        nc.tensor.matmul(
                            psum,
                            wbuf[:, ko, dl * 128 : dl * 128 + 128],
                            srcT[:, ko, 0:512],
                            start=(ko == 0),
                            stop=(ko == 7),
                        )
                    nc.vector.tensor_scalar_add(
                        out=qt[:, dt, :], in0=psum, scalar1=bq2[:, dt : dt + 1]
                    )

            # ---- V projection (natural layout + ones column), bf16 ----
            v_sb = persist.tile([128, 8, 16, 65], BF16, tag="v_sb")
            for mt in range(8):
                nc.gpsimd.dma_start(
                    out=v_sb[:, mt, :, 64:65],
                    in_=_bcast_ap(OnesB[0, 0:1], [128, 16]),
                )
            for vc in range(4):
                wbuf = wpool.tile([128, 8, 256], F32R, tag="wbuf")
                nc.sync.dma_start(
                    out=wbuf,
                    in_=Wv[:, vc * 256 : vc * 256 + 256].rearrange(
                        "(ko ki) d -> ki ko d", ki=128
                    ),
                )
                for mt in range(8):
                    psum = ps_mm.tile([128, 512], F32, tag="psmm")
                    nc.tensor.matmul(
                        psum[:, 0:256],
                        ones_row[0:1, 0:128],
                        bv_r[0:1, vc * 256 : vc * 256 + 256],
                        start=True,
                        stop=False,
                    )
                    for ko in range(8):
                        nc.tensor.matmul(
                            psum[:, 0:256],
                            srcT[:, ko, mt * 128 : mt * 128 + 128],
                            wbuf[:, ko, :],
                            start=False,
                            stop=(ko == 7),
                        )
                    nc.vector.tensor_copy(
                        out=v_sb[:, mt, vc * 4 : vc * 4 + 4, 0:64],
                        in_=psum[:, 0:256].rearrange("p (h e) -> p h e", e=64),
                    )

            # ---- attention, head by head ----
            ao_sb = persist.tile([128, 8, 512], BF16, tag="ao_sb")
            for h in range(H):
                base = (h % 2) * 64
                dt = h // 2
                pao = ps_mm.tile([65, 512], F32, tag="psmm")
                for kog in range(4):
                    pss = ps_s.tile([128, 2, 512], F32, tag="pss")
                    for kl in range(2):
                        ko = kog * 2 + kl
                        nc.tensor.matmul(
                            pss[:, kl, :], ident, biasT[:, ko, :],
                            start=True, stop=False,
                        )
                        nc.tensor.matmul(
                            pss[:, kl, :],
                            kt[base : base + 64, dt, ko * 128 : ko * 128 + 128],
                            qt[base : base + 64, dt, :],
                            start=False,
                            stop=True,
                        )
                    e_t = epool.tile([128, 2, 512], BF16, tag="e_t")
                    nc.scalar.activation(out=e_t, in_=pss, func=FT.Exp)
                    for kl in range(2):
                        ko = kog * 2 + kl
                        nc.tensor.matmul(
                            pao,
                            v_sb[:, ko, h, :],
                            e_t[:, kl, :],
                            start=(ko == 0),
                            stop=(ko == 7),
                        )
                rec = small.tile([1, 512], F32, tag="rec")
                nc.vector.reciprocal(out=rec, in_=pao[64:65, :])
                drec = dscratch.tile([1, 512], F32, tag="drec")
                nc.sync.dma_start(out=drec, in_=rec)
                R_sb = small.tile([64, 512], F32, tag="R_sb")
                nc.sync.dma_start(out=R_sb, in_=_bcast_ap(drec[0, :], [64]))
                nc.vector.tensor_mul(
                    out=ao_sb[base : base + 64, dt, :], in0=pao[0:64, :], in1=R_sb
                )

            # ---- out projection + residual; x = LN1(src + ao@Wo + bo) ----
            src_q = arena2b.tile([128, 4, 1024], F32, tag="a2b")
            for qt_i in range(4):
                nc.sync.dma_start(
                    out=src_q[:, qt_i, :],
                    in_=SrcQ[qt_i * 128 : qt_i * 128 + 128, :],
                )
            x_sb = persist.tile([128, 4, 1024], F32R, tag="x_sb")
            xT = arena2.tile([128, 8, 512], F32R, tag="a2")
            xpre_all = arena4b.tile([128, 4, 1024], F32, tag="a4b")
            for nt in range(2):
                wobuf = wpool.tile([128, 8, 512], BF16, tag="wbuf")
                nc.sync.dma_start(
                    out=wobuf,
                    in_=Wo[:, nt * 512 : nt * 512 + 512].rearrange(
                        "(dp ki) d -> ki dp d", ki=128
                    ),
                )
                for qt_i in range(4):
                    psum = ps_mm.tile([128, 512], F32, tag="psmm")
                    for dpi in range(8):
                        nc.tensor.matmul(
                            psum,
                            ao_sb[:, dpi, qt_i * 128 : qt_i * 128 + 128],
                            wobuf[:, dpi, :],
                            start=(dpi == 0),
                            stop=(dpi == 7),
                        )
                    nc.vector.tensor_add(
                        out=xpre_all[:, qt_i, nt * 512 : nt * 512 + 512],
                        in0=psum,
                        in1=src_q[:, qt_i, nt * 512 : nt * 512 + 512],
                    )
            for qt_i in range(4):
                xpre = xpre_all[:, qt_i, :]
                # LayerNorm 1
                stats = small.tile([128, 2, 6], F32, tag="stats")
                for half in range(2):
                    nc.vector.bn_stats(
                        out=stats[:, half, :],
                        in_=xpre[:, half * 512 : half * 512 + 512],
                    )
                mv = small.tile([128, 2], F32, tag="mv")
                nc.vector.bn_aggr(out=mv, in_=stats)
                sq = small.tile([128, 1], F32, tag="sq")
                nc.scalar.activation(
                    out=sq, in_=mv[:, 1:2], func=FT.Sqrt, bias=ln_eps
                )
                rstd = small.tile([128, 1], F32, tag="rstd")
                nc.vector.reciprocal(out=rstd, in_=sq)
                nmr = small.tile([128, 1], F32, tag="nmr")
                nc.vector.tensor_scalar(
                    out=nmr,
                    in0=mv[:, 0:1],
                    scalar1=rstd,
                    scalar2=-1.0,
                    op0=ALU.mult,
                    op1=ALU.mult,
                )
                xn = lnpool.tile([128, 1024], F32, tag="lnbig2")
                nc.scalar.activation(
                    out=xn, in_=xpre, func=FT.Identity, bias=nmr, scale=rstd
                )
                xg = lnpool.tile([128, 1024], F32, tag="lnbig")
                nc.gpsimd.tensor_mul(out=xg, in0=xn, in1=g1b)
                nc.vector.tensor_add(out=x_sb[:, qt_i, :], in0=xg, in1=be1b)
                for ct in range(8):
                    pt = ps_mm.tile([128, 512], F32R, tag="psmm")
                    nc.tensor.transpose(
                        pt[:, 0:128],
                        x_sb[:, qt_i, ct * 128 : ct * 128 + 128],
                        ident,
                    )
                    nc.vector.tensor_copy(
                        out=xT[:, ct, qt_i * 128 : qt_i * 128 + 128],
                        in_=pt[:, 0:128],
                    )

            # ---- FFN mm1 + relu: h[f, q] bf16 ----
            h_sb = arena4.tile([128, 32, 512], BF16, tag="a4")
            for fc in range(16):
                wbuf = wpool.tile([128, 8, 256], F32R, tag="wbuf")
                nc.sync.dma_start(
                    out=wbuf,
                    in_=W1[:, fc * 256 : fc * 256 + 256].rearrange(
                        "(co ki) f -> ki co f", ki=128
                    ),
                )
                for fl in range(2):
                    ft = fc * 2 + fl
                    psum = ps_mm.tile([128, 512], F32, tag="psmm")
                    for co in range(8):
                        nc.tensor.matmul(
                            psum,
                            wbuf[:, co, fl * 128 : fl * 128 + 128],
                            xT[:, co, :],
                            start=(co == 0),
                            stop=(co == 7),
                        )
                    nc.scalar.activation(
                        out=h_sb[:, ft, :],
                        in_=psum,
                        func=FT.Relu,
                        bias=b12[:, ft : ft + 1],
                    )

            # ---- FFN mm2 + residual; out = LN2(x + h@W2 + b2) ----
            ypre_all = arena2b.tile([128, 4, 1024], F32, tag="a2b")
            for nt in range(2):
                w2buf = arena4b.tile([128, 32, 512], BF16, tag="a4b")
                nc.sync.dma_start(
                    out=w2buf,
                    in_=W2[:, nt * 512 : nt * 512 + 512].rearrange(
                        "(fo fi) d -> fi fo d", fi=128
                    ),
                )
                for qt_i in range(4):
                    psum = ps_mm.tile([128, 512], F32, tag="psmm")
                    nc.tensor.matmul(
                        psum,
                        ones_row[0:1, 0:128],
                        b2_r[0:1, nt * 512 : nt * 512 + 512],
                        start=True,
                        stop=False,
                    )
                    for ft in range(32):
                        nc.tensor.matmul(
                            psum,
                            h_sb[:, ft, qt_i * 128 : qt_i * 128 + 128],
                            w2buf[:, ft, :],
                            start=False,
                            stop=(ft == 31),
                        )
                    nc.vector.tensor_add(
                        out=ypre_all[:, qt_i, nt * 512 : nt * 512 + 512],
                        in0=psum,
                        in1=x_sb[:, qt_i, nt * 512 : nt * 512 + 512],
                    )

            g2b = consts.tile([128, D], BF16, tag="gb")
            nc.sync.dma_start(out=g2b, in_=_bcast_ap(G2[:], [128]))
            be2b = consts.tile([128, D], BF16, tag="beb")
            nc.sync.dma_start(out=be2b, in_=_bcast_ap(Be2[:], [128]))
            for qt_i in range(4):
                ypre = ypre_all[:, qt_i, :]
                stats = small.tile([128, 2, 6], F32, tag="stats")
                for half in range(2):
                    nc.vector.bn_stats(
                        out=stats[:, half, :],
                        in_=ypre[:, half * 512 : half * 512 + 512],
                    )
                mv = small.tile([128, 2], F32, tag="mv")
                nc.vector.bn_aggr(out=mv, in_=stats)
                sq = small.tile([128, 1], F32, tag="sq")
                nc.scalar.activation(
                    out=sq, in_=mv[:, 1:2], func=FT.Sqrt, bias=ln_eps
                )
                rstd = small.tile([128, 1], F32, tag="rstd")
                nc.vector.reciprocal(out=rstd, in_=sq)
                nmr = small.tile([128, 1], F32, tag="nmr")
                nc.vector.tensor_scalar(
                    out=nmr,
                    in0=mv[:, 0:1],
                    scalar1=rstd,
                    scalar2=-1.0,
                    op0=ALU.mult,
                    op1=ALU.mult,
                )
                yn = lnpool.tile([128, 1024], F32, tag="lnbig")
                nc.scalar.activation(
                    out=yn, in_=ypre, func=FT.Identity, bias=nmr, scale=rstd
                )
                yg = lnpool.tile([128, 1024], F32, tag="lnbig2")
                nc.gpsimd.tensor_mul(out=yg, in0=yn, in1=g2b)
                out_t = lnpool.tile([128, 1024], F32, tag="lnbig")
                nc.vector.tensor_add(out=out_t, in0=yg, in1=be2b)
                nc.sync.dma_start(
                    out=Out[qt_i * 128 : qt_i * 128 + 128, :], in_=out_t
                )

    _legalize_waits(nc)
    return nc


_CACHE = {}


def kernel(**inputs):
    import os

    if "nc" not in _CACHE:
        _CACHE["nc"] = _build()
    nc = _CACHE["nc"]

    f32 = np.float32
    bf16 = ml_dtypes.bfloat16
    src = np.asarray(inputs["src"], f32)
    distances = np.asarray(inputs["distances"], f32)
    scale = np.float32(HD ** -0.5)
    Wq_s = (np.asarray(inputs["Wq"], f32) * scale).astype(f32)
    bq_s = (np.asarray(inputs["bq"], f32) * scale).astype(f32)
    Wk = np.ascontiguousarray(np.asarray(inputs["Wk"], f32))
    Wv = np.ascontiguousarray(np.asarray(inputs["Wv"], f32))
    Wo = np.asarray(inputs["Wo"], f32).astype(bf16)
    W1 = np.ascontiguousarray(np.asarray(inputs["W1"], f32))
    W2 = np.asarray(inputs["W2"], f32).astype(bf16)

    shared = {
        "Wq": Wq_s,
        "Wk": Wk,
        "Wv": Wv,
        "Wo": Wo,
        "W1": W1,
        "W2": W2,
        "bq2d": np.ascontiguousarray(bq_s.reshape(8, 128).T),
        "bk2d": np.ascontiguousarray(np.asarray(inputs["bk"], f32).reshape(8, 128).T),
        "b1_2d": np.ascontiguousarray(np.asarray(inputs["b1"], f32).reshape(32, 128).T),
        "bv_r": np.asarray(inputs["bv"], f32).reshape(1, D).copy(),
        "b2_r": np.asarray(inputs["b2"], f32).reshape(1, D).copy(),
        "g1v": np.asarray(inputs["g1"], f32).astype(bf16),
        "beta1v": np.asarray(inputs["beta1"], f32).astype(bf16),
        "g2v": np.asarray(inputs["g2"], f32).astype(bf16),
        "beta2v": np.asarray(inputs["beta2"], f32).astype(bf16),
        "negabs": np.array([[-abs(float(np.asarray(inputs["dist_scale"])))]], f32),
        "ident": np.eye(128, dtype=f32),
        "ones_row": np.ones((1, 512), f32),
        "ones_bf": np.ones((1, 1), bf16),
    }

    in_maps = []
    for c in range(NCORES):
        b, qh = c // 2, c % 2
        q0 = qh * SQ
        if qh == 0:
            perm = np.arange(S)
        else:
            perm = np.r_[np.arange(512, 1024), np.arange(0, 512)]
        m = dict(shared)
        m["srcT"] = np.ascontiguousarray(src[b][perm].T)
        m["src_q"] = np.ascontiguousarray(src[b, q0 : q0 + SQ] + np.asarray(inputs["bo"], f32)[None, :])
        m["distT"] = np.ascontiguousarray(distances[b, q0 : q0 + SQ][:, perm].T)
        in_maps.append(m)

    trace = bool(int(os.environ.get("BASS_KERNEL_TRACE", "0")))
    res = run_bass_kernel_spmd(
        nc,
        in_maps,
        core_ids=list(range(NCORES)),
        trace=trace,
        stitch_traces=False,
    )
    _CACHE["last_result"] = res

    out = np.empty((B, S, D), f32)
    for c in range(NCORES):
        b, qh = c // 2, c % 2
        out[b, qh * SQ : qh * SQ + SQ] = res.results[c]["out"]
    return out



# revision 5
# speedup vs baseline: 1.0854x; 1.0854x over previous
"""Distance-aware transformer encoder layer on 8 Trainium2 NeuronCores.

Sharding: core c handles batch b = c//2 and query-half qh = c%2 (512 query
rows). K/V are computed per-core for the full 1024-key sequence of its batch
(duplicated across the core pair — cheaper than a collective). Everything
else (scores, softmax, out-proj, LayerNorms, FFN) is perfectly sharded by
query rows. No collectives.

Precision: bf16 operands for every matmul (fp32 PSUM accumulation); fp32
residual path (src_q, xpre, x, ypre) and LayerNorm statistics. The distance
bias is folded in exp-space: host ships P^T = (d+1e-9)^(-|dist_scale|) as
bf16 and the kernel computes E = exp(scores) * P on Scalar+Vector, which
removes the per-head identity-matmul bias injection of the earlier version.
Softmax normalization broadcasts 1/rowsum via a 1-row PE matmul instead of
a DRAM round-trip. W2 streams through SBUF once (8 PSUM banks accumulate
all 4 query tiles x 2 output halves simultaneously).
"""

import numpy as np
import ml_dtypes

import bass_rust
import concourse.bass as bass
import concourse.tile as tile
import concourse.mybir as mybir
from concourse.bass import AP
from concourse.bass_utils import run_bass_kernel_spmd

B, S, D, H, DFF, HD = 4, 1024, 1024, 16, 4096, 64
SQ = 512          # query rows per core
NCORES = 8
EPS = 1e-5
F32 = mybir.dt.float32
F32R = mybir.dt.float32r
BF16 = mybir.dt.bfloat16
FT = mybir.ActivationFunctionType
ALU = mybir.AluOpType

_nop_ctr = [0]


def _legalize_waits(nc):
    """walrus codegen in this toolchain accepts only one sync-wait per
    instruction; split extras onto same-engine NoOps inserted before."""
    n_fixed = 0
    for f in nc.m.functions:
        for bb in f.blocks:
            insts = bb.instructions
            i = 0
            while i < len(insts):
                inst = insts[i]
                si = inst.sync_info
                waits = list(si.on_wait) if si is not None and si.on_wait else []
                if len(waits) > 1:
                    keep = waits[-1]
                    for w in waits[:-1]:
                        n = bass_rust.InstNoOp(
                            name=f"waitsplit-nop-{_nop_ctr[0]}", ins=[], outs=[]
                        )
                        _nop_ctr[0] += 1
                        n.engine = inst.engine
                        n.sync_info = bass_rust.SyncInfo(on_update=[], on_wait=[w])
                        insts.insert(i, n)
                        i += 1
                    inst.sync_info = bass_rust.SyncInfo(
                        on_update=list(si.on_update or []), on_wait=[keep]
                    )
                    n_fixed += 1
                i += 1
    return n_fixed


def _bcast_ap(dram_ap, reps):
    """Broadcast a DRAM AP over leading zero-step dims of sizes `reps`."""
    return AP(
        tensor=dram_ap.tensor,
        offset=dram_ap.offset,
        ap=[[0, r] for r in reps] + list(dram_ap.ap),
    )


def _build():
    nc = bass.Bass()
    dp = nc.declare_dram_parameter

    SrcT = dp("srcT", [S, S], BF16, isOutput=False)        # src[b][perm].T
    SrcQ = dp("src_q", [SQ, D], F32, isOutput=False)       # natural query rows (+bo)
    PTp = dp("pt_bias", [S, SQ], BF16, isOutput=False)     # (d+1e-9)^(-|s|), keys x q
    Wq = dp("Wq", [D, D], BF16, isOutput=False)            # pre-scaled by HD^-0.5
    Wk = dp("Wk", [D, D], BF16, isOutput=False)
    Wv = dp("Wv", [D, D], BF16, isOutput=False)
    Wo = dp("Wo", [D, D], BF16, isOutput=False)
    W1 = dp("W1", [D, DFF], BF16, isOutput=False)
    W2 = dp("W2", [DFF, D], BF16, isOutput=False)
    Bq2 = dp("bq2d", [128, 8], F32, isOutput=False)        # (bq*scale) tiled
    Bk2 = dp("bk2d", [128, 8], F32, isOutput=False)
    B12 = dp("b1_2d", [128, 32], F32, isOutput=False)
    BvR = dp("bv_r", [1, D], BF16, isOutput=False)
    B2R = dp("b2_r", [1, D], BF16, isOutput=False)
    G1 = dp("g1v", [D], BF16, isOutput=False)
    Be1 = dp("beta1v", [D], BF16, isOutput=False)
    G2 = dp("g2v", [D], BF16, isOutput=False)
    Be2 = dp("beta2v", [D], BF16, isOutput=False)
    Ident = dp("ident", [128, 128], F32R, isOutput=False)
    OnesB = dp("ones_brow", [1, 128], BF16, isOutput=False)
    OnesF = dp("ones_f64", [1, 64], F32R, isOutput=False)
    OnesS = dp("ones_bf", [1, 1], BF16, isOutput=False)
    Out = dp("out", [SQ, D], F32, isOutput=True)

    with tile.TileContext(nc) as tc:
        import contextlib

        ctx = contextlib.ExitStack()
        with ctx:
            consts = ctx.enter_context(tc.tile_pool(name="consts", bufs=1))
            big1 = ctx.enter_context(tc.tile_pool(name="big1", bufs=1))
            big2 = ctx.enter_context(tc.tile_pool(name="big2", bufs=1))
            big3 = ctx.enter_context(tc.tile_pool(name="big3", bufs=1))
            ptp = ctx.enter_context(tc.tile_pool(name="ptp", bufs=1))
            vpool = ctx.enter_context(tc.tile_pool(name="vpool", bufs=1))
            resq = ctx.enter_context(tc.tile_pool(name="resq", bufs=1))
            xpool = ctx.enter_context(tc.tile_pool(name="xpool", bufs=1))
            wpool = ctx.enter_context(tc.tile_pool(name="wpool", bufs=2))
            w1pool = ctx.enter_context(tc.tile_pool(name="w1pool", bufs=3))
            w2pool = ctx.enter_context(tc.tile_pool(name="w2pool", bufs=2))
            epool = ctx.enter_context(tc.tile_pool(name="epool", bufs=3))
            lnpool = ctx.enter_context(tc.tile_pool(name="lnpool", bufs=1))
            small = ctx.enter_context(tc.tile_pool(name="small", bufs=2))
            ps = ctx.enter_context(tc.tile_pool(name="ps", bufs=1, space="PSUM"))

            # ---- constants ----
            ident = consts.tile([128, 128], F32R, tag="ident")
            nc.sync.dma_start(out=ident, in_=Ident[:, :])
            ones_b = consts.tile([1, 128], BF16, tag="ones_b")
            nc.sync.dma_start(out=ones_b, in_=OnesB[:, :])
            ones_f = consts.tile([1, 64], F32R, tag="ones_f")
            nc.sync.dma_start(out=ones_f, in_=OnesF[:, :])
            bq2 = consts.tile([128, 8], F32, tag="bq2")
            nc.sync.dma_start(out=bq2, in_=Bq2[:, :])
            bk2 = consts.tile([128, 8], F32, tag="bk2")
            nc.sync.dma_start(out=bk2, in_=Bk2[:, :])
            b12 = consts.tile([128, 32], F32, tag="b12")
            nc.sync.dma_start(out=b12, in_=B12[:, :])
            bv_r = consts.tile([1, D], BF16, tag="bv_r")
            nc.sync.dma_start(out=bv_r, in_=BvR[:, :])
            b2_r = consts.tile([1, D], BF16, tag="b2_r")
            nc.sync.dma_start(out=b2_r, in_=B2R[:, :])
            g1b = consts.tile([128, D], BF16, tag="g1b")
            nc.sync.dma_start(out=g1b, in_=_bcast_ap(G1[:], [128]))
            be1b = consts.tile([128, D], BF16, tag="be1b")
            nc.sync.dma_start(out=be1b, in_=_bcast_ap(Be1[:], [128]))
            ln_eps = consts.tile([128, 1], F32, tag="ln_eps")
            nc.vector.memset(ln_eps, EPS)

            # ---- srcT resident (keys on free dim, viewed [128, 2co+nt, 512]) ----
            st = big1.tile([128, 32, 512], BF16, tag="big1", name="st")
            for co in range(8):
                nc.sync.dma_start(
                    out=st[:, 2 * co : 2 * co + 2, :],
                    in_=SrcT[co * 128 : co * 128 + 128, :].rearrange(
                        "p (a m) -> p a m", a=2
                    ),
                )

            # small PE spin to cover initial DMA ramp
            spin = ps.tile([128, 512], F32, tag="gen", bufs=2, name="spin")
            for _ in range(8):
                nc.tensor.matmul(spin[:, 0:128], ident, ident, start=True, stop=True)

            def gen_psum(i, name="p"):
                return ps.tile(
                    [128, 512], F32, tag=("gen" if i % 2 == 0 else "pao"),
                    bufs=2, name=name,
                )

            # ---- K^T projection: kt[dout, keys] bf16 ----
            kt = big2.tile([128, 8, 1024], BF16, tag="big2", name="kt")
            pctr = 0
            for wc in range(2):
                wkbuf = wpool.tile([128, 8, 512], BF16, tag="w", name="wkbuf")
                nc.sync.dma_start(
                    out=wkbuf,
                    in_=Wk[:, wc * 512 : wc * 512 + 512].rearrange(
                        "(ko ki) m -> ki ko m", ki=128
                    ),
                )
                for dl in range(4):
                    dt = wc * 4 + dl
                    for nt in range(2):
                        psum = gen_psum(pctr); pctr += 1
                        for ko in range(8):
                            nc.tensor.matmul(
                                psum,
                                wkbuf[:, ko, dl * 128 : dl * 128 + 128],
                                st[:, 2 * ko + nt, :],
                                start=(ko == 0),
                                stop=(ko == 7),
                            )
                        nc.vector.tensor_scalar_add(
                            out=kt[:, dt, nt * 512 : nt * 512 + 512],
                            in0=psum,
                            scalar1=bk2[:, dt : dt + 1],
                        )

            # ---- Q^T projection: qt[dout, q] bf16 (Wq pre-scaled) ----
            qt = big3.tile([128, 8, 512], BF16, tag="big3", name="qt")
            for wc in range(2):
                wqbuf = wpool.tile([128, 8, 512], BF16, tag="w", name="wqbuf")
                nc.sync.dma_start(
                    out=wqbuf,
                    in_=Wq[:, wc * 512 : wc * 512 + 512].rearrange(
                        "(ko ki) m -> ki ko m", ki=128
                    ),
                )
                for dl in range(4):
                    dt = wc * 4 + dl
                    psum = gen_psum(pctr); pctr += 1
                    for ko in range(8):
                        nc.tensor.matmul(
                            psum,
                            wqbuf[:, ko, dl * 128 : dl * 128 + 128],
                            st[:, 2 * ko, :],
                            start=(ko == 0),
                            stop=(ko == 7),
                        )
                    nc.vector.tensor_scalar_add(
                        out=qt[:, dt, :], in0=psum, scalar1=bq2[:, dt : dt + 1]
                    )

            # ---- V projection (natural layout + ones column), bf16 ----
            v_sb = vpool.tile([128, 8, 16, 65], BF16, tag="v_sb")
            for mt in range(8):
                nc.gpsimd.dma_start(
                    out=v_sb[:, mt, :, 64:65],
                    in_=_bcast_ap(OnesS[0, 0:1], [128, 16]),
                )
            for vc in range(2):
                wvbuf = wpool.tile([128, 8, 512], BF16, tag="w", name="wvbuf")
                nc.sync.dma_start(
                    out=wvbuf,
                    in_=Wv[:, vc * 512 : vc * 512 + 512].rearrange(
                        "(ko ki) d -> ki ko d", ki=128
                    ),
                )
                for mt in range(8):
                    psum = gen_psum(pctr); pctr += 1
                    nc.tensor.matmul(
                        psum,
                        ones_b[0:1, 0:128],
                        bv_r[0:1, vc * 512 : vc * 512 + 512],
                        start=True,
                        stop=False,
                    )
                    for ko in range(8):
                        nc.tensor.matmul(
                            psum,
                            st[:, 2 * ko + mt // 4, (mt % 4) * 128 : (mt % 4) * 128 + 128],
                            wvbuf[:, ko, :],
                            start=False,
                            stop=(ko == 7),
                        )
                    nc.vector.tensor_copy(
                        out=v_sb[:, mt, vc * 8 : vc * 8 + 8, 0:64],
                        in_=psum.rearrange("p (h e) -> p h e", e=64),
                    )

            # ---- prefetch for post-attention phases (DMA-idle window) ----
            pt_sb = ptp.tile([128, 8, 512], BF16, tag="ptx", name="pt_sb")
            for ko in range(8):
                nc.sync.dma_start(
                    out=pt_sb[:, ko, :], in_=PTp[ko * 128 : ko * 128 + 128, :]
                )
            g2b = consts.tile([128, D], BF16, tag="g2b")
            nc.sync.dma_start(out=g2b, in_=_bcast_ap(G2[:], [128]))
            be2b = consts.tile([128, D], BF16, tag="be2b")
            nc.sync.dma_start(out=be2b, in_=_bcast_ap(Be2[:], [128]))
            src_q = resq.tile([128, 4, 1024], F32, tag="resq", name="src_q")
            for qt_i in range(4):
                nc.sync.dma_start(
                    out=src_q[:, qt_i, :],
                    in_=SrcQ[qt_i * 128 : qt_i * 128 + 128, :],
                )
            wobufs = []
            for nt in range(2):
                wobuf = wpool.tile([128, 8, 512], BF16, tag="w", name="wobuf")
                nc.sync.dma_start(
                    out=wobuf,
                    in_=Wo[:, nt * 512 : nt * 512 + 512].rearrange(
                        "(dp ki) d -> ki dp d", ki=128
                    ),
                )
                wobufs.append(wobuf)

            def load_w1(fc):
                w1buf = w1pool.tile([128, 8, 512], BF16, tag="w1", name="w1buf")
                nc.sync.dma_start(
                    out=w1buf,
                    in_=W1[:, fc * 512 : fc * 512 + 512].rearrange(
                        "(ko ki) f -> ki ko f", ki=128
                    ),
                )
                return w1buf

            def load_w2(c):
                w2c = w2pool.tile([128, 4, 1024], BF16, tag="w2", name="w2c")
                nc.sync.dma_start(
                    out=w2c,
                    in_=W2[c * 512 : c * 512 + 512, :].rearrange(
                        "(fo fi) d -> fi fo d", fi=128
                    ),
                )
                return w2c

            w1bufs = [load_w1(0), load_w1(1), load_w1(2)]
            w2cs = [load_w2(0), load_w2(1)]

            # ---- attention, head by head; ao written during attention into
            # the big1 slot (srcT is dead once V is done) ----
            ao_sb = big1.tile([128, 8, 512], BF16, tag="big1", name="ao_sb")
            for h in range(H):
                base = (h % 2) * 64
                dt = h // 2
                pao = ps.tile([128, 512], F32, tag="pao", bufs=2, name="pao")
                for kog in range(4):
                    pss = ps.tile([128, 2, 512], F32, tag="pss", bufs=2, name="pss")
                    for kl in range(2):
                        ko = kog * 2 + kl
                        nc.tensor.matmul(
                            pss[:, kl, :],
                            kt[base : base + 64, dt, ko * 128 : ko * 128 + 128],
                            qt[base : base + 64, dt, :],
                            start=True,
                            stop=True,
                        )
                    e_raw = epool.tile([128, 2, 512], BF16, tag="e_t", name="e_raw")
                    nc.scalar.activation(out=e_raw, in_=pss, func=FT.Exp)
                    e_t = epool.tile([128, 2, 512], BF16, tag="e_t", name="e_t")
                    nc.vector.tensor_mul(
                        out=e_t, in0=e_raw, in1=pt_sb[:, 2 * kog : 2 * kog + 2, :]
                    )
                    for kl in range(2):
                        ko = kog * 2 + kl
                        nc.tensor.matmul(
                            pao[0:65, :],
                            v_sb[:, ko, h, :],
                            e_t[:, kl, :],
                            start=(ko == 0),
                            stop=(ko == 7),
                        )
                # normalize: PE-broadcast the sums row to 64 partitions, then
                # a parallel 64-lane reciprocal (scalar Reciprocal is blocked)
                s_row = small.tile([1, 512], F32R, tag="s_row")
                nc.vector.tensor_copy(out=s_row, in_=pao[64:65, :])
                psb = ps.tile([128, 512], F32, tag="gen", bufs=2, name="psb")
                nc.tensor.matmul(
                    psb[0:64, :], ones_f[0:1, :], s_row, start=True, stop=True
                )
                rcpT = small.tile([64, 512], F32, tag="rcpT")
                nc.vector.reciprocal(out=rcpT, in_=psb[0:64, :])
                nc.vector.tensor_mul(
                    out=ao_sb[base : base + 64, dt, :], in0=pao[0:64, :], in1=rcpT
                )

            # ---- out projection + residual; x = LN1(src + ao@Wo + bo) ----
            x_sb = xpool.tile([128, 4, 1024], F32R, tag="x_sb")
            xT = ptp.tile([128, 8, 512], BF16, tag="ptx", name="xT")
            xpre_all = big2.tile([128, 4, 1024], F32, tag="big2", name="xpre_all")
            for nt in range(2):
                for qt_i in range(4):
                    psum = gen_psum(pctr); pctr += 1
                    for dpi in range(8):
                        nc.tensor.matmul(
                            psum,
                            ao_sb[:, dpi, qt_i * 128 : qt_i * 128 + 128],
                            wobufs[nt][:, dpi, :],
                            start=(dpi == 0),
                            stop=(dpi == 7),
                        )
                    nc.vector.tensor_add(
                        out=xpre_all[:, qt_i, nt * 512 : nt * 512 + 512],
                        in0=psum,
                        in1=src_q[:, qt_i, nt * 512 : nt * 512 + 512],
                    )
            for qt_i in range(4):
                xpre = xpre_all[:, qt_i, :]
                # LayerNorm 1
                stats = small.tile([128, 2, 6], F32, tag="stats")
                for half in range(2):
                    nc.vector.bn_stats(
                        out=stats[:, half, :],
                        in_=xpre[:, half * 512 : half * 512 + 512],
                    )
                mv = small.tile([128, 2], F32, tag="mv")
                nc.vector.bn_aggr(out=mv, in_=stats)
                sq = small.tile([128, 1], F32, tag="sq")
                nc.scalar.activation(
                    out=sq, in_=mv[:, 1:2], func=FT.Sqrt, bias=ln_eps
                )
                rstd = small.tile([128, 1], F32, tag="rstd")
                nc.vector.reciprocal(out=rstd, in_=sq)
                nmr = small.tile([128, 1], F32, tag="nmr")
                nc.vector.tensor_scalar(
                    out=nmr,
                    in0=mv[:, 0:1],
                    scalar1=rstd,
                    scalar2=-1.0,
                    op0=ALU.mult,
                    op1=ALU.mult,
                )
                xn = lnpool.tile([128, 1024], F32, tag="lnbig2")
                nc.scalar.activation(
                    out=xn, in_=xpre, func=FT.Identity, bias=nmr, scale=rstd
                )
                xg = lnpool.tile([128, 1024], F32, tag="lnbig")
                nc.gpsimd.tensor_mul(out=xg, in0=xn, in1=g1b)
                nc.vector.tensor_add(out=x_sb[:, qt_i, :], in0=xg, in1=be1b)
                for ct in range(8):
                    pt = ps.tile(
                        [128, 512], F32R,
                        tag=("gen" if ct % 2 == 0 else "pao"), bufs=2, name="pt",
                    )
                    nc.tensor.transpose(
                        pt[:, 0:128],
                        x_sb[:, qt_i, ct * 128 : ct * 128 + 128],
                        ident,
                    )
                    nc.vector.tensor_copy(
                        out=xT[:, ct, qt_i * 128 : qt_i * 128 + 128],
                        in_=pt[:, 0:128],
                    )

            # ---- FFN mm1 + relu: h[f, q] bf16 (big1 slot again) ----
            h_sb = big1.tile([128, 32, 512], BF16, tag="big1", name="h_sb")
            for fc in range(8):
                w1buf = w1bufs[fc] if fc < 3 else load_w1(fc)
                for fl in range(4):
                    ft = fc * 4 + fl
                    psum = gen_psum(pctr); pctr += 1
                    for ko in range(8):
                        nc.tensor.matmul(
                            psum,
                            w1buf[:, ko, fl * 128 : fl * 128 + 128],
                            xT[:, ko, :],
                            start=(ko == 0),
                            stop=(ko == 7),
                        )
                    nc.scalar.activation(
                        out=h_sb[:, ft, :],
                        in_=psum,
                        func=FT.Relu,
                        bias=b12[:, ft : ft + 1],
                    )

            # ---- FFN mm2 single-pass W2 stream; all 8 PSUM banks accumulate
            # (4 query tiles x 2 output halves); out = LN2(x + h@W2 + b2) ----
            pfA = ps.tile([128, 2, 512], F32, tag="pss", bufs=2, name="pfA")
            pfB = ps.tile([128, 2, 512], F32, tag="pss", bufs=2, name="pfB")
            pfC = ps.tile([128, 512], F32, tag="pao", bufs=2, name="pfC")
            pfD = ps.tile([128, 512], F32, tag="pao", bufs=2, name="pfD")
            pfE = ps.tile([128, 512], F32, tag="gen", bufs=2, name="pfE")
            pfF = ps.tile([128, 512], F32, tag="gen", bufs=2, name="pfF")
            psf = [
                pfA[:, 0, :], pfA[:, 1, :],
                pfB[:, 0, :], pfB[:, 1, :],
                pfC, pfD,
                pfE, pfF,
            ]
            for qt_i in range(4):
                for nt in range(2):
                    nc.tensor.matmul(
                        psf[qt_i * 2 + nt],
                        ones_b[0:1, 0:128],
                        b2_r[0:1, nt * 512 : nt * 512 + 512],
                        start=True,
                        stop=False,
                    )
            for c in range(8):
                w2c = w2cs[c] if c < 2 else load_w2(c)
                for j in range(4):
                    ft = c * 4 + j
                    for qt_i in range(4):
                        for nt in range(2):
                            nc.tensor.matmul(
                                psf[qt_i * 2 + nt],
                                h_sb[:, ft, qt_i * 128 : qt_i * 128 + 128],
                                w2c[:, j, nt * 512 : nt * 512 + 512],
                                start=False,
                                stop=(c == 7 and j == 3),
                            )

            ypre_all = resq.tile([128, 4, 1024], F32, tag="resq", name="ypre_all")
            for qt_i in range(4):
                for nt in range(2):
                    nc.vector.tensor_add(
                        out=ypre_all[:, qt_i, nt * 512 : nt * 512 + 512],
                        in0=psf[qt_i * 2 + nt],
                        in1=x_sb[:, qt_i, nt * 512 : nt * 512 + 512],
                    )
                ypre = ypre_all[:, qt_i, :]
                stats = small.tile([128, 2, 6], F32, tag="stats")
                for half in range(2):
                    nc.vector.bn_stats(
                        out=stats[:, half, :],
                        in_=ypre[:, half * 512 : half * 512 + 512],
                    )
                mv = small.tile([128, 2], F32, tag="mv")
                nc.vector.bn_aggr(out=mv, in_=stats)
                sq = small.tile([128, 1], F32, tag="sq")
                nc.scalar.activation(
                    out=sq, in_=mv[:, 1:2], func=FT.Sqrt, bias=ln_eps
                )
                rstd = small.tile([128, 1], F32, tag="rstd")
                nc.vector.reciprocal(out=rstd, in_=sq)
                nmr = small.tile([128, 1], F32, tag="nmr")
                nc.vector.tensor_scalar(
                    out=nmr,
                    in0=mv[:, 0:1],
                    scalar1=rstd,
                    scalar2=-1.0,
                    op0=ALU.mult,
                    op1=ALU.mult,
                )
                yn = lnpool.tile([128, 1024], F32, tag="lnbig")
                nc.scalar.activation(
                    out=yn, in_=ypre, func=FT.Identity, bias=nmr, scale=rstd
                )
                yg = lnpool.tile([128, 1024], F32, tag="lnbig2")
                nc.gpsimd.tensor_mul(out=yg, in0=yn, in1=g2b)
                out_t = lnpool.tile([128, 1024], F32, tag="lnbig")
                nc.vector.tensor_add(out=out_t, in0=yg, in1=be2b)
                nc.sync.dma_start(
                    out=Out[qt_i * 128 : qt_i * 128 + 128, :], in_=out_t
                )

    _legalize_waits(nc)
    return nc


_CACHE = {}


def kernel(**inputs):
    import os

    if "nc" not in _CACHE:
        _CACHE["nc"] = _build()
    nc = _CACHE["nc"]

    f32 = np.float32
    bf16 = ml_dtypes.bfloat16
    src = np.asarray(inputs["src"], f32)
    distances = np.asarray(inputs["distances"], f32)
    scale = np.float32(HD ** -0.5)
    Wq_s = (np.asarray(inputs["Wq"], f32) * scale).astype(bf16)
    bq_s = (np.asarray(inputs["bq"], f32) * scale).astype(f32)
    Wk = np.asarray(inputs["Wk"], f32).astype(bf16)
    Wv = np.asarray(inputs["Wv"], f32).astype(bf16)
    Wo = np.asarray(inputs["Wo"], f32).astype(bf16)
    W1 = np.asarray(inputs["W1"], f32).astype(bf16)
    W2 = np.asarray(inputs["W2"], f32).astype(bf16)
    nabs = abs(float(np.asarray(inputs["dist_scale"])))

    shared = {
        "Wq": Wq_s,
        "Wk": Wk,
        "Wv": Wv,
        "Wo": Wo,
        "W1": W1,
        "W2": W2,
        "bq2d": np.ascontiguousarray(bq_s.reshape(8, 128).T),
        "bk2d": np.ascontiguousarray(np.asarray(inputs["bk"], f32).reshape(8, 128).T),
        "b1_2d": np.ascontiguousarray(np.asarray(inputs["b1"], f32).reshape(32, 128).T),
        "bv_r": np.asarray(inputs["bv"], f32).reshape(1, D).astype(bf16),
        "b2_r": np.asarray(inputs["b2"], f32).reshape(1, D).astype(bf16),
        "g1v": np.asarray(inputs["g1"], f32).astype(bf16),
        "beta1v": np.asarray(inputs["beta1"], f32).astype(bf16),
        "g2v": np.asarray(inputs["g2"], f32).astype(bf16),
        "beta2v": np.asarray(inputs["beta2"], f32).astype(bf16),
        "ident": np.eye(128, dtype=f32),
        "ones_brow": np.ones((1, 128), bf16),
        "ones_f64": np.ones((1, 64), f32),
        "ones_bf": np.ones((1, 1), bf16),
    }

    in_maps = []
    for c in range(NCORES):
        b, qh = c // 2, c % 2
        q0 = qh * SQ
        if qh == 0:
            perm = np.arange(S)
        else:
            perm = np.r_[np.arange(512, 1024), np.arange(0, 512)]
        m = dict(shared)
        m["srcT"] = np.ascontiguousarray(src[b][perm].T).astype(bf16)
        m["src_q"] = np.ascontiguousarray(
            src[b, q0 : q0 + SQ] + np.asarray(inputs["bo"], f32)[None, :]
        )
        dT = np.ascontiguousarray(distances[b, q0 : q0 + SQ][:, perm].T)
        m["pt_bias"] = np.exp(np.log(dT + np.float32(1e-9)) * np.float32(-nabs)).astype(bf16)
        in_maps.append(m)

    trace = bool(int(os.environ.get("BASS_KERNEL_TRACE", "0")))
    res = run_bass_kernel_spmd(
        nc,
        in_maps,
        core_ids=list(range(NCORES)),
        trace=trace,
        stitch_traces=False,
    )
    _CACHE["last_result"] = res

    out = np.empty((B, S, D), f32)
    for c in range(NCORES):
        b, qh = c // 2, c % 2
        out[b, qh * SQ : qh * SQ + SQ] = res.results[c]["out"]
    return out
